# revision 28
# baseline (speedup 1.0000x reference)
"""Trainium2 Bass kernel for the SNN Leaky-Integrate-Fire problem.

Pipeline (per core, pure data-parallel over batch):
  cur1 = x @ W1.T + b1                        [B,32]
  100x: mem = beta*mem + cur1 - H(mem-1)      (elementwise scan)
  spk  = H(mem - 1)
  out  = spk @ W2.T + b2                      [B,3]

Everything runs on the DVE. The scan uses a custom DVE op (per-NEFF
micro-op table, see concourse/dve_ops.py) that fuses TWO full LIF steps
into one instruction:
  h0 = (m > 1); m' = (m*beta + c) - h0
  h1 = (m' > 1); m'' = (m'*beta + c) - h1
Each ALU stage rounds fp32 exactly like the reference's
fl(fl(fl(beta*m)+c)-h) sequence, so the scan is bit-exact. A second
custom op fuses the final step with the spike threshold. Steps 4..99
ride ONE instruction via a stride-0 repeat dim ([P, 48, 2048] views of
the state): each pass re-streams the same columns, and because the read
stream trails the write stream by a whole 2048-element pass (vs ~124
cycles of write-commit latency), pass r+1 reads pass r's output. This
amortizes the ~230ns custom-op issue cost over 96 steps.

The head (K=3 matmul) is 6 tensor_tensor ops with stride-0 broadcast
views (the PE's fp32 path is ~40us for this shape; DVE does it in ~14us
and keeps everything on one engine with no cross-engine sync), split in
two halves so the second half's input DMA overlaps the first half's
compute. The tail uses a third custom op (prefix-sum of spk*w2o) plus
one strided STT per output that differences block boundaries and adds
the bias, so fc2 is one full-width pass per output instead of two.

Layout per core: 8192 rows; logical row r = chunk*128 + p lives at
partition p, free block chunk. Host feeds x packed as [128, 64*3] and
inverse-permutes the output rows.
"""
import sys

sys.path.insert(0, "/opt/trn_rl_repo")

import numpy as np

import concourse.bacc as bacc
import concourse.tile as tile
from concourse import dve_ops, mybir
from concourse.bass_utils import run_bass_kernel_spmd
from concourse.dve_spec import AluOp, Spec, Src0, Src1, C0, C1, lower, scan
from concourse.dve_spec import _has_src1 as has_src1
from concourse.dve_uop import DveOpSpec

F32 = mybir.dt.float32
ALU = mybir.AluOpType

# problem constants (hardcoded per contract)
B, N_IN, N_HID, N_OUT = 65536, 3, 32, 3
NUM_STEPS, BETA, THR = 100, 0.9, 1.0
N_CORES = 8
BC = B // N_CORES          # rows per core = 8192
P = 128                    # partitions
NCH = BC // P              # 128-row chunks per core = 64
FREE = NCH * N_HID         # scan free size = 2048

# const block layout (replicated across partitions):
# [b1(32) w2(3*32) b2(3) pad w1k(3*32)]
B1_OFF, W2_OFF, B2_OFF, W1_OFF = 0, 32, 128, 160
WB_COLS = 256


def _register_op(name, spec):
    """Append a custom DVE op to dve_ops.OPS (the documented extension
    point) with a self-computed uops sha. Idempotent per process."""
    for op in dve_ops.OPS:
        if op.name == name:
            return op
    row = dve_ops._CUSTOM_DVE_ROW_BASE + len(dve_ops.OPS)
    shas = {}
    for ver in ("v3", "v4"):
        s = DveOpSpec(
            name=name, opcode=row, uops=lower(spec, ver=ver),
            rd1_en=has_src1(spec),
        )
        shas[ver] = s.sha(ver)
    op = dve_ops.DveOp(name, spec, subdim=False, uops_sha=shas)
    dve_ops.OPS.append(op)
    dve_ops.CUSTOM_DVE_SPECS[name] = spec
    dve_ops._SUB_OPCODE_FOR_NAME[name] = row
    return op


def _lif2_ref(in0, in1, s0, s1, imm2):
    b, t = np.float32(s0), np.float32(s1)
    m1 = (in0 * b + in1) - (in0 > t).astype(np.float32)
    return (m1 * b + in1) - (m1 > t).astype(np.float32)


def _liff_ref(in0, in1, s0, s1, imm2):
    b, t = np.float32(s0), np.float32(s1)
    m1 = (in0 * b + in1) - (in0 > t).astype(np.float32)
    return (m1 > t).astype(np.float32)


def _mulscan_ref(in0, in1, s0, s1, imm2):
    p = (in0.astype(np.float32) * in1.astype(np.float32)).reshape(in0.shape[0], -1)
    return np.cumsum(p, axis=1, dtype=np.float32).reshape(in0.shape)


_m1 = (Src0 * C0 + Src1) - (Src0 > C1)
LIF2 = _register_op(
    "LIF2_ANT",
    Spec(body=(_m1 * C0 + Src1) - (_m1 > C1), reference=_lif2_ref),
)
LIFF = _register_op(
    "LIF_FINAL_ANT",
    Spec(body=_m1 > C1, reference=_liff_ref),
)
MULSCAN = _register_op(
    "MULSCAN_ANT",
    Spec(body=scan(AluOp.ADD, Src0 * Src1), reference=_mulscan_ref),
)


def build(nc, n_rows_core=BC, num_steps=NUM_STEPS):
    nch = n_rows_core // P
    free = nch * N_HID
    assert num_steps % 2 == 0  # m1 = c seed, then (num_steps-2)/2 LIF2 + LIFF

    nh = nch // 2  # chunks per half
    xa_d = nc.dram_tensor("xpa", [P, nh * N_IN], F32, kind="ExternalInput")
    xb_d = nc.dram_tensor("xpb", [P, nh * N_IN], F32, kind="ExternalInput")
    wb_d = nc.dram_tensor("wb", [P, WB_COLS], F32, kind="ExternalInput")
    y_d = nc.dram_tensor("y", [n_rows_core, N_OUT], F32, kind="ExternalOutput")

    y_view = y_d[:].rearrange("(p i) o -> p (i o)", p=P)

    dve = nc.vector

    with tile.TileContext(nc) as tc:
        with tc.tile_pool(name="pool", bufs=1) as pool:
            xta_t = pool.tile([P, nh * N_IN], F32, tag="xta")
            xtb = pool.tile([P, nh * N_IN], F32, tag="xtb")
            wt_t = pool.tile([P, WB_COLS], F32, tag="wt")
            nc.scalar.dma_start(wt_t[:], wb_d[:])
            nc.scalar.dma_start(xta_t[:], xa_d[:])
            nc.sync.dma_start(xtb[:], xb_d[:])
            wt = wt_t[:]
            xta = xta_t[:]

            ct = pool.tile([P, free], F32, tag="ct")   # cur1
            mt = pool.tile([P, free], F32, tag="mt")   # mem state
            at = pool.tile([P, free], F32, tag="at")   # scratch / spikes
            pt = pool.tile([P, N_OUT * free + N_HID], F32, tag="pt")  # prefixes
            ot = pool.tile([P, nch * N_OUT], F32, tag="ot")
            # W2 replicated per chunk (o-major); DMA'd during the scan.
            w2r_d = nc.dram_tensor("w2r", [P, N_OUT * free], F32,
                                   kind="ExternalInput")
            w2t = pool.tile([P, N_OUT * free], F32, tag="w2t")
            nc.sync.dma_start(w2t[:], w2r_d[:])

            def cbc(off, blocks=nch):
                # [P, 32] const slice -> [P, blocks, 32] broadcast view
                return (
                    wt[:, off : off + N_HID]
                    .unsqueeze(1)
                    .broadcast_to([P, blocks, N_HID])
                )

            def h3(ap):
                return ap.rearrange("p (i h) -> p i h", h=N_HID)

            # --- head: cur1 = x @ W1.T + b1 as 6 broadcast TT ops/half ---
            hw = nh * N_HID  # cols per half
            for xv, c0 in (
                (xta.rearrange("p (i k) -> p i k", k=N_IN), 0),
                (xtb[:].rearrange("p (i k) -> p i k", k=N_IN), hw),
            ):

                def xk(k):
                    return xv[:, :, k : k + 1].broadcast_to([P, nh, N_HID])

                cs = ct[:, c0 : c0 + hw]
                as_ = at[:, c0 : c0 + hw]
                dve.tensor_tensor(h3(cs), xk(0), cbc(W1_OFF, nh), ALU.mult)
                dve.tensor_tensor(h3(as_), xk(1), cbc(W1_OFF + N_HID, nh), ALU.mult)
                dve.tensor_tensor(cs, cs, as_, ALU.add)
                dve.tensor_tensor(h3(as_), xk(2), cbc(W1_OFF + 2 * N_HID, nh), ALU.mult)
                dve.tensor_tensor(cs, cs, as_, ALU.add)
                dve.tensor_tensor(h3(cs), h3(cs), cbc(B1_OFF, nh), ALU.add)

            # --- scan: m1 = cur1; fused double steps; final step + spike ---
            nc.vector._custom_dve(
                LIF2, out=mt[:], in0=ct[:], in1=ct[:], s0=BETA, s1=THR
            )
            # steps 4..num_steps-1 ride ONE instruction: a stride-0 repeat dim
            # streams the state through the op R times (the read stream trails
            # the write stream by a whole 2048-elem pass, far beyond the ~124
            # cycle write-commit latency, so pass r+1 reads pass r's output).
            # Amortizes the ~230ns custom-op issue cost over 2R steps.
            R = (num_steps - 4) // 2
            mrep = mt[:].unsqueeze(1).broadcast_to([P, R, free])
            crep = ct[:].unsqueeze(1).broadcast_to([P, R, free])
            nc.vector._custom_dve(
                LIF2, out=mrep, in0=mrep, in1=crep, s0=BETA, s1=THR
            )
            nc.vector._custom_dve(
                LIFF, out=at[:], in0=mt[:], in1=ct[:], s0=BETA, s1=THR
            )

            # --- fc2: y = spk @ W2.T + b2 via ONE prefix-sum pass ---
            # spk repeats x3 (stride-0) against the replicated W2 table; the
            # scan accumulator runs continuously across all 3*2048 positions —
            # carried offsets cancel in the block-boundary differences.
            # pt[0] = 0; pt[1+j] = prefix(spk*w2)[j], j = o*2048 + i*32 + h;
            # block (o,i) = pt[o*2048+32(i+1)] - pt[o*2048+32i]; per-o STT
            # differences and adds b2.
            dve.memset(pt[:, 0:1], 0.0)
            nc.vector._custom_dve(
                MULSCAN,
                out=pt[:, 1 : 1 + N_OUT * free].rearrange(
                    "p (o c) -> p o c", o=N_OUT
                ),
                in0=at[:].unsqueeze(1).broadcast_to([P, N_OUT, free]),
                in1=w2t[:].rearrange("p (o c) -> p o c", o=N_OUT),
            )
            ovi = ot[:].rearrange("p (i o) -> p i o", o=N_OUT)
            for o in range(N_OUT):
                lo_v = pt[:, o * free : (o + 1) * free].rearrange(
                    "p (i h) -> p i h", h=N_HID
                )
                hi_v = pt[:, o * free + N_HID : (o + 1) * free + N_HID].rearrange(
                    "p (i h) -> p i h", h=N_HID
                )
                dve.scalar_tensor_tensor(
                    ovi[:, :, o : o + 1],
                    hi_v[:, :, 0:1],
                    wt[:, B2_OFF + o : B2_OFF + o + 1],
                    lo_v[:, :, 0:1],
                    ALU.add,
                    ALU.subtract,
                )

            nc.sync.dma_start(y_view, ot[:])
    return nc


_CACHE = {}


def _get_program():
    if "nc" not in _CACHE:
        nc = bacc.Bacc("TRN2", target_bir_lowering=False, debug=False,
                       num_devices=N_CORES)
        build(nc)
        nc.compile()
        _CACHE["nc"] = nc
    return _CACHE["nc"]


def make_wb(b1, W2, b2, W1):
    wb = np.zeros((P, WB_COLS), dtype=np.float32)
    wb[:, B1_OFF : B1_OFF + N_HID] = b1
    wb[:, W2_OFF : W2_OFF + N_HID * N_OUT] = np.ascontiguousarray(W2).reshape(-1)
    wb[:, B2_OFF : B2_OFF + N_OUT] = b2
    for k in range(N_IN):
        wb[:, W1_OFF + N_HID * k : W1_OFF + N_HID * (k + 1)] = W1[:, k]
    return wb


def kernel(x, W1, b1, W2, b2):
    x = np.asarray(x, dtype=np.float32)
    W1, b1, W2, b2 = (np.asarray(a, dtype=np.float32) for a in (W1, b1, W2, b2))
    wb = make_wb(b1, W2, b2, W1)
    # W2 replicated o-major per chunk: w2r[p, o*2048 + i*32 + h] = W2[o, h]
    w2r = np.ascontiguousarray(np.broadcast_to(
        np.tile(W2.reshape(N_OUT, 1, N_HID), (1, NCH, 1)).reshape(-1),
        (P, N_OUT * FREE),
    ))
    nc = _get_program()
    in_maps = []
    for i in range(N_CORES):
        xs = x[i * BC : (i + 1) * BC]
        # row r = ch*128 + p  ->  x_sb[p, ch*3 + k]
        xp = xs.reshape(NCH, P, N_IN).transpose(1, 0, 2).reshape(P, NCH * N_IN)
        hc = NCH // 2 * N_IN
        in_maps.append({
            "xpa": np.ascontiguousarray(xp[:, :hc]),
            "xpb": np.ascontiguousarray(xp[:, hc:]),
            "wb": wb,
            "w2r": w2r,
        })
    kwargs = dict(_CACHE.get("run_kwargs") or {})
    res = run_bass_kernel_spmd(nc, in_maps, core_ids=list(range(N_CORES)), **kwargs)
    _CACHE["last_results"] = res
    # y rows are stored permuted: dram row p*NCH + ch  <->  logical row ch*P + p
    out = np.empty((B, N_OUT), dtype=np.float32)
    for i in range(N_CORES):
        yc = res.results[i]["y"].reshape(P, NCH, N_OUT)
        out[i * BC : (i + 1) * BC] = yc.transpose(1, 0, 2).reshape(BC, N_OUT)
    return out


# revision 32
# speedup vs baseline: 1.0382x; 1.0382x over previous
"""Trainium2 Bass kernel for the SNN Leaky-Integrate-Fire problem.

Pipeline (per core, pure data-parallel over batch):
  cur1 = x @ W1.T + b1                        [B,32]
  100x: mem = beta*mem + cur1 - H(mem-1)      (elementwise scan)
  spk  = H(mem - 1)
  out  = spk @ W2.T + b2                      [B,3]

Everything runs on the DVE. The scan uses a custom DVE op (per-NEFF
micro-op table, see concourse/dve_ops.py) that fuses TWO full LIF steps
into one instruction:
  h0 = (m > 1); m' = (m*beta + c) - h0
  h1 = (m' > 1); m'' = (m'*beta + c) - h1
Each ALU stage rounds fp32 exactly like the reference's
fl(fl(fl(beta*m)+c)-h) sequence, so the scan is bit-exact. A second
custom op fuses the final step with the spike threshold. Steps 4..99
ride ONE instruction via a stride-0 repeat dim ([P, 48, 2048] views of
the state): each pass re-streams the same columns, and because the read
stream trails the write stream by a whole 2048-element pass (vs ~124
cycles of write-commit latency), pass r+1 reads pass r's output. This
amortizes the ~230ns custom-op issue cost over 96 steps.

The head (K=3 matmul) is 6 tensor_tensor ops with stride-0 broadcast
views (the PE's fp32 path is ~40us for this shape; DVE does it in ~14us
and keeps everything on one engine with no cross-engine sync), split in
two halves so the second half's input DMA overlaps the first half's
compute. The tail uses a third custom op (prefix-sum of spk*w2o) plus
one strided STT per output that differences block boundaries and adds
the bias, so fc2 is one full-width pass per output instead of two.

Layout per core: 8192 rows; logical row r = chunk*128 + p lives at
partition p, free block chunk. Host feeds x packed as [128, 64*3] and
inverse-permutes the output rows.
"""
import sys

sys.path.insert(0, "/opt/trn_rl_repo")

import numpy as np

import concourse.bacc as bacc
import concourse.tile as tile
from concourse import dve_ops, mybir
from concourse.bass_utils import run_bass_kernel_spmd
from concourse.dve_spec import AluOp, Spec, Src0, Src1, C0, C1, lower, scan
from concourse.dve_spec import _has_src1 as has_src1
from concourse.dve_uop import DveOpSpec

F32 = mybir.dt.float32
ALU = mybir.AluOpType

# problem constants (hardcoded per contract)
B, N_IN, N_HID, N_OUT = 65536, 3, 32, 3
NUM_STEPS, BETA, THR = 100, 0.9, 1.0
N_CORES = 8
BC = B // N_CORES          # rows per core = 8192
P = 128                    # partitions
NCH = BC // P              # 128-row chunks per core = 64
FREE = NCH * N_HID         # scan free size = 2048

# const block layout (replicated across partitions):
# [b1(32) w2(3*32) b2(3) pad w1k(3*32)]
B1_OFF, W2_OFF, B2_OFF, W1_OFF = 0, 32, 128, 160
WB_COLS = 256


def _register_op(name, spec):
    """Append a custom DVE op to dve_ops.OPS (the documented extension
    point) with a self-computed uops sha. Idempotent per process."""
    for op in dve_ops.OPS:
        if op.name == name:
            return op
    row = dve_ops._CUSTOM_DVE_ROW_BASE + len(dve_ops.OPS)
    shas = {}
    for ver in ("v3", "v4"):
        s = DveOpSpec(
            name=name, opcode=row, uops=lower(spec, ver=ver),
            rd1_en=has_src1(spec),
        )
        shas[ver] = s.sha(ver)
    op = dve_ops.DveOp(name, spec, subdim=False, uops_sha=shas)
    dve_ops.OPS.append(op)
    dve_ops.CUSTOM_DVE_SPECS[name] = spec
    dve_ops._SUB_OPCODE_FOR_NAME[name] = row
    return op


def _lif2_ref(in0, in1, s0, s1, imm2):
    b, t = np.float32(s0), np.float32(s1)
    m1 = (in0 * b + in1) - (in0 > t).astype(np.float32)
    return (m1 * b + in1) - (m1 > t).astype(np.float32)


def _liff_ref(in0, in1, s0, s1, imm2):
    b, t = np.float32(s0), np.float32(s1)
    m1 = (in0 * b + in1) - (in0 > t).astype(np.float32)
    return (m1 > t).astype(np.float32)


def _mulscan_ref(in0, in1, s0, s1, imm2):
    p = (in0.astype(np.float32) * in1.astype(np.float32)).reshape(in0.shape[0], -1)
    return np.cumsum(p, axis=1, dtype=np.float32).reshape(in0.shape)


_m1 = (Src0 * C0 + Src1) - (Src0 > C1)
LIF2 = _register_op(
    "LIF2_ANT",
    Spec(body=(_m1 * C0 + Src1) - (_m1 > C1), reference=_lif2_ref),
)
LIFF = _register_op(
    "LIF_FINAL_ANT",
    Spec(body=_m1 > C1, reference=_liff_ref),
)
MULSCAN = _register_op(
    "MULSCAN_ANT",
    Spec(body=scan(AluOp.ADD, Src0 * Src1), reference=_mulscan_ref),
)


def build(nc, n_rows_core=BC, num_steps=NUM_STEPS):
    nch = n_rows_core // P
    free = nch * N_HID
    assert num_steps % 2 == 0  # m1 = c seed, then (num_steps-2)/2 LIF2 + LIFF

    nh = nch // 2  # chunks per half
    xa_d = nc.dram_tensor("xpa", [P, nh * N_IN], F32, kind="ExternalInput")
    xb_d = nc.dram_tensor("xpb", [P, nh * N_IN], F32, kind="ExternalInput")
    wb_d = nc.dram_tensor("wb", [P, WB_COLS], F32, kind="ExternalInput")
    y_d = nc.dram_tensor("y", [n_rows_core, N_OUT], F32, kind="ExternalOutput")

    y_view = y_d[:].rearrange("(p i) o -> p (i o)", p=P)

    dve = nc.vector

    with tile.TileContext(nc) as tc:
        with tc.tile_pool(name="pool", bufs=1) as pool:
            xta_t = pool.tile([P, nh * N_IN], F32, tag="xta")
            xtb = pool.tile([P, nh * N_IN], F32, tag="xtb")
            wt_t = pool.tile([P, WB_COLS], F32, tag="wt")
            nc.scalar.dma_start(wt_t[:], wb_d[:])
            nc.scalar.dma_start(xta_t[:], xa_d[:])
            nc.sync.dma_start(xtb[:], xb_d[:])
            wt = wt_t[:]
            xta = xta_t[:]

            ct = pool.tile([P, free], F32, tag="ct")   # cur1
            mt = pool.tile([P, free], F32, tag="mt")   # mem state
            at = pool.tile([P, free], F32, tag="at")   # scratch / spikes
            pt = pool.tile([P, free + N_HID], F32, tag="pt")  # prefix sums
            ot = pool.tile([P, nch * N_OUT], F32, tag="ot")

            def cbc(off, blocks=nch):
                # [P, 32] const slice -> [P, blocks, 32] broadcast view
                return (
                    wt[:, off : off + N_HID]
                    .unsqueeze(1)
                    .broadcast_to([P, blocks, N_HID])
                )

            def h3(ap):
                return ap.rearrange("p (i h) -> p i h", h=N_HID)

            # --- head: cur1 = x @ W1.T + b1 as 6 broadcast TT ops/half ---
            hw = nh * N_HID  # cols per half
            for xv, c0 in (
                (xta.rearrange("p (i k) -> p i k", k=N_IN), 0),
                (xtb[:].rearrange("p (i k) -> p i k", k=N_IN), hw),
            ):

                def xk(k):
                    return xv[:, :, k : k + 1].broadcast_to([P, nh, N_HID])

                cs = ct[:, c0 : c0 + hw]
                as_ = at[:, c0 : c0 + hw]
                dve.tensor_tensor(h3(cs), xk(0), cbc(W1_OFF, nh), ALU.mult)
                dve.tensor_tensor(h3(as_), xk(1), cbc(W1_OFF + N_HID, nh), ALU.mult)
                dve.tensor_tensor(cs, cs, as_, ALU.add)
                dve.tensor_tensor(h3(as_), xk(2), cbc(W1_OFF + 2 * N_HID, nh), ALU.mult)
                dve.tensor_tensor(cs, cs, as_, ALU.add)
                dve.tensor_tensor(h3(cs), h3(cs), cbc(B1_OFF, nh), ALU.add)

            # --- scan: m1 = cur1; fused double steps; final step + spike ---
            nc.vector._custom_dve(
                LIF2, out=mt[:], in0=ct[:], in1=ct[:], s0=BETA, s1=THR
            )
            # steps 4..num_steps-1 ride ONE instruction: a stride-0 repeat dim
            # streams the state through the op R times (the read stream trails
            # the write stream by a whole 2048-elem pass, far beyond the ~124
            # cycle write-commit latency, so pass r+1 reads pass r's output).
            # Amortizes the ~230ns custom-op issue cost over 2R steps.
            R = (num_steps - 4) // 2
            mrep = mt[:].unsqueeze(1).broadcast_to([P, R, free])
            crep = ct[:].unsqueeze(1).broadcast_to([P, R, free])
            nc.vector._custom_dve(
                LIF2, out=mrep, in0=mrep, in1=crep, s0=BETA, s1=THR
            )
            nc.vector._custom_dve(
                LIFF, out=at[:], in0=mt[:], in1=ct[:], s0=BETA, s1=THR
            )

            # --- fc2: y = spk @ W2.T + b2 via prefix sums ---
            # pt[0] = 0; pt[1+k] = prefix_sum(spk*w2o)[k]; block sum b =
            # pt[32(b+1)] - pt[32b]; one strided STT adds b2 and differences.
            dve.memset(pt[:, 0:1], 0.0)
            ovi = ot[:].rearrange("p (i o) -> p i o", o=N_OUT)
            lo_v = pt[:, 0:free].rearrange("p (i h) -> p i h", h=N_HID)
            hi_v = pt[:, N_HID : N_HID + free].rearrange("p (i h) -> p i h", h=N_HID)
            for o in range(N_OUT):
                nc.vector._custom_dve(
                    MULSCAN,
                    out=h3(pt[:, 1 : 1 + free]),
                    in0=h3(at[:]),
                    in1=cbc(W2_OFF + N_HID * o),
                )
                dve.scalar_tensor_tensor(
                    ovi[:, :, o : o + 1],
                    hi_v[:, :, 0:1],
                    wt[:, B2_OFF + o : B2_OFF + o + 1],
                    lo_v[:, :, 0:1],
                    ALU.add,
                    ALU.subtract,
                )

            nc.sync.dma_start(y_view, ot[:])
    return nc


_CACHE = {}


def _get_program():
    if "nc" not in _CACHE:
        nc = bacc.Bacc("TRN2", target_bir_lowering=False, debug=False,
                       num_devices=N_CORES)
        build(nc)
        nc.compile()
        _CACHE["nc"] = nc
    return _CACHE["nc"]


def make_wb(b1, W2, b2, W1):
    wb = np.zeros((P, WB_COLS), dtype=np.float32)
    wb[:, B1_OFF : B1_OFF + N_HID] = b1
    wb[:, W2_OFF : W2_OFF + N_HID * N_OUT] = np.ascontiguousarray(W2).reshape(-1)
    wb[:, B2_OFF : B2_OFF + N_OUT] = b2
    for k in range(N_IN):
        wb[:, W1_OFF + N_HID * k : W1_OFF + N_HID * (k + 1)] = W1[:, k]
    return wb


def kernel(x, W1, b1, W2, b2):
    x = np.asarray(x, dtype=np.float32)
    W1, b1, W2, b2 = (np.asarray(a, dtype=np.float32) for a in (W1, b1, W2, b2))
    wb = make_wb(b1, W2, b2, W1)
    nc = _get_program()
    in_maps = []
    for i in range(N_CORES):
        xs = x[i * BC : (i + 1) * BC]
        # row r = ch*128 + p  ->  x_sb[p, ch*3 + k]
        xp = xs.reshape(NCH, P, N_IN).transpose(1, 0, 2).reshape(P, NCH * N_IN)
        hc = NCH // 2 * N_IN
        in_maps.append({
            "xpa": np.ascontiguousarray(xp[:, :hc]),
            "xpb": np.ascontiguousarray(xp[:, hc:]),
            "wb": wb,
        })
    kwargs = dict(_CACHE.get("run_kwargs") or {})
    res = run_bass_kernel_spmd(nc, in_maps, core_ids=list(range(N_CORES)), **kwargs)
    _CACHE["last_results"] = res
    # y rows are stored permuted: dram row p*NCH + ch  <->  logical row ch*P + p
    out = np.empty((B, N_OUT), dtype=np.float32)
    for i in range(N_CORES):
        yc = res.results[i]["y"].reshape(P, NCH, N_OUT)
        out[i * BC : (i + 1) * BC] = yc.transpose(1, 0, 2).reshape(BC, N_OUT)
    return out
